# revision 36
# baseline (speedup 1.0000x reference)
"""Multi-head dilated sliding-window attention (window=129, dil=1) on 8 TRN2 cores.

Sharding: sequence-parallel. Each core computes 256 query rows (N=2048 / 8),
with a 64-row K/V halo on each side (zero-padded at the sequence edges).
Weights are replicated (resident in SBUF, bf16).

Band-softmax identity used (reference softmaxes the FULL row with zeros
outside the band):
    out_i = (sum_band (e^{s_ij} - 1) V_j + sum_all V_j) / (sum_band (e^{s_ij} - 1) + N)
computed per head with V_raw = x@Wv (no bias; bv is folded in after the
attention average). bk is applied unconditionally as a per-partition scalar
bias; contributions from zero-padded halo keys (which would wrongly score
e^{q.bk}-1 != 0) are killed by per-core masks that zero those key rows.
The global row sum_all V_j = (sum_n x_n) @ Wv and its +N denominator count
are precomputed on the host into `biascat`.

Compute dtype: bf16 operands into the PE (fp32 runs at quarter rate on TRN2),
fp32 PSUM accumulation; the exp/-1/mask chain runs in bf16 on ACT/DVE (2x DVE
rate; P is stored bf16 anyway so this costs no extra error).

Structure: Q^T/K^T projections are computed per head-pair (db) and attention
for that pair runs immediately, pipelined one round behind the scores so the
PE never stalls on the ACT/DVE softmax chain. The output projection
O = (A + bv) @ Wo is folded into the same pipeline: each round transposes the
previous pair's attention rows and accumulates A_db @ Wo[db-slice] into 4
persistent PSUM banks, so only the bias row + output DMA remain at the end.
"""

import numpy as np
import ml_dtypes
from contextlib import ExitStack

import concourse.bass as bass
import concourse.tile as tile
from concourse import bacc, mybir
from concourse.bass_utils import run_bass_kernel_spmd

F32 = mybir.dt.float32
BF16 = mybir.dt.bfloat16
NPBF16 = ml_dtypes.bfloat16
N, E, H, D = 2048, 1024, 16, 64
R = N // 8          # 256 query rows per core
HALO = R + 128      # 384 K/V rows per core
NQB = R // 128      # query blocks per core


def build_graph():
    nc = bacc.Bacc("TRN2", target_bir_lowering=False, debug=False, num_devices=8)

    xh_d = nc.declare_dram_parameter("xhT", [E, HALO], BF16, isOutput=False)
    wq_d = nc.declare_dram_parameter("Wq", [E, H * D], BF16, isOutput=False)
    wk_d = nc.declare_dram_parameter("Wk", [E, H * D], BF16, isOutput=False)
    wv_d = nc.declare_dram_parameter("Wv", [E, H * D], BF16, isOutput=False)
    wo_d = nc.declare_dram_parameter("Wo", [H * D, E], BF16, isOutput=False)
    bqk_d = nc.declare_dram_parameter("bqk_r", [128, 16], F32, isOutput=False)
    bcbo_d = nc.declare_dram_parameter("bcbo", [1, H * (D + 1) + E], BF16,
                                       isOutput=False)
    m4_d = nc.declare_dram_parameter("mask4", [128, 512], BF16, isOutput=False)
    id_d = nc.declare_dram_parameter("ident", [128, 128], BF16, isOutput=False)
    out_d = nc.declare_dram_parameter("out", [R, E], BF16, isOutput=True)

    with tile.TileContext(nc) as tc, ExitStack() as ctx:
        const = ctx.enter_context(tc.tile_pool(name="const", bufs=1))
        pers = ctx.enter_context(tc.tile_pool(name="pers", bufs=1))
        epool = ctx.enter_context(tc.tile_pool(name="epool", bufs=3))
        ppool = ctx.enter_context(tc.tile_pool(name="ppool", bufs=5))
        zpool = ctx.enter_context(tc.tile_pool(name="zpool", bufs=4))
        atpool = ctx.enter_context(tc.tile_pool(name="atpool", bufs=3))
        obpool = ctx.enter_context(tc.tile_pool(name="obpool", bufs=2))
        psum = ctx.enter_context(tc.tile_pool(name="psum", bufs=4, space="PSUM"))
        opsum = ctx.enter_context(tc.tile_pool(name="opsum", bufs=1, space="PSUM"))

        def ps(shape, dt=F32):
            return psum.tile(shape, dt, tag="ps", name="pst")

        # ---- loads. A dma_start costs ~607ns of issue time on the Sync
        # engine REGARDLESS of size, and each transfer's packets spray
        # round-robin over all 16 DMA engines (~432 GB/s aggregate). So:
        # fewest possible dma_starts, ordered by first use; weights are
        # split in two chunks only so consumers can start on the first half.
        # x arrives pre-transposed from the host: [e_p, e_t, seq]
        xT = pers.tile([128, 8, HALO], BF16, tag="xT")
        identity = const.tile([128, 128], BF16, tag="identity")

        def wtile(nm):
            return const.tile([128, 8, E], BF16, tag=nm, name="wt")

        wv_t = wtile("wv")
        wq_t = wtile("wq")
        wk_t = wtile("wk")
        wo_t = wtile("wo")
        m4 = const.tile([128, 512], BF16, tag="m4")
        bqk_sb = const.tile([128, 16], F32, tag="bqk")
        bq_sb = bqk_sb[:, 0:8]
        bk_sb = bqk_sb[:, 8:16]
        bcbo_sb = const.tile([1, H * (D + 1) + E], BF16, tag="bcbo")
        biascat = bcbo_sb[:, 0:H * (D + 1)].rearrange(
            "o (h d) -> o h d", d=D + 1)
        bo_sb = bcbo_sb[:, H * (D + 1):]

        # DMA order = v9 tuning: xhT whole, wv in 4 chunks (V ramps the PE
        # clock on its DMA-paced prefix, which also keeps early activity
        # density below the HAM duty-throttle trip point), then wq/wk/wo.
        # Starting V earlier (interleaved xhT/wv chunks) measured WORSE:
        # the denser early activity trips a 4/8 duty clamp onto rounds 0-1.
        nc.sync.dma_start(
            xT[:], xh_d[:, :].rearrange("(et p) s -> p et s", p=128))
        nc.sync.dma_start(identity[:], id_d[:, :])
        wvsrc = wv_d[:, :].rearrange("(et p) c -> p et c", p=128)
        for ch in range(4):
            nc.sync.dma_start(wv_t[:, 2 * ch:2 * ch + 2, :],
                              wvsrc[:, 2 * ch:2 * ch + 2, :])
        nc.sync.dma_start(m4[:], m4_d[:, :])
        nc.sync.dma_start(bqk_sb[:], bqk_d[:, :])
        nc.sync.dma_start(bcbo_sb[:], bcbo_d[:, :])

        def wdma(wt, src_d):
            src = src_d[:, :].rearrange("(et p) c -> p et c", p=128)
            nc.sync.dma_start(wt[:, 0:4, :], src[:, 0:4, :])
            nc.sync.dma_start(wt[:, 4:8, :], src[:, 4:8, :])

        wdma(wq_t, wq_d)
        wdma(wk_t, wk_d)
        wdma(wo_t, wo_d)
        ones_sb = const.tile([1, 128], BF16, tag="ones")
        nc.vector.memset(ones_sb[:], 1.0)

        # ---- persistent activations ---------------------------------------
        QT = pers.tile([128, 8, R], BF16, tag="QT")          # [d_p, d_t, q]
        KT = pers.tile([128, 8, HALO], BF16, tag="KT")       # [d_p, d_t, seq]
        Vaug = pers.tile([128, 3, H, D + 1], BF16, tag="Vaug")
        Asc = pers.tile([128, NQB, H * D], BF16, tag="Asc")  # [q_p, qblk, dims]

        # ---- persistent O-projection accumulators (4 PSUM banks) ----------
        ops = [opsum.tile([128, 512], F32, tag=f"ops{i}", name=f"ops{i}")
               for i in range(2 * NQB)]

        # ---- V (natural layout, raw): st-serial so only 2 PSUM banks are
        # held, letting the first attention rounds overlap later V blocks.
        for st in range(3):
            vp = [ps([128, 512]) for _ in range(2)]
            for et in range(8):
                for hf in range(2):
                    nc.tensor.matmul(vp[hf][:],
                                     xT[:, et, st * 128:(st + 1) * 128],
                                     wv_t[:, et, hf * 512:(hf + 1) * 512],
                                     start=(et == 0), stop=(et == 7))
            for hf in range(2):
                src = vp[hf][:].rearrange("p (h d) -> p h d", d=D)
                nc.scalar.copy(Vaug[:, st, hf * 8:(hf + 1) * 8, 0:D], src)
        nc.vector.memset(Vaug[:, :, :, D:D + 1], 1.0)

        # ---- fused projections + banded attention + O accumulation, one
        # head-pair at a time. Round r = db. Emission order:
        #   1. Q^T/K^T projection matmuls for db
        #   2. PV + bias matmuls, epilogue, A-transpose and O-accumulation
        #      of round r-1 (p tiles ready)
        #   3. S matmuls (one [128, 512] psum per head = both qblk/cblk
        #      quadrants) + exp/-1/mask chain for round r
        # Per-head p layout: [q0c0 | q0c1 | q1c0 | q1c1], quadrant j uses
        # keys halo block (qblk+cblk) and mask m0/m1 alternating.
        prev = None  # (db, ptiles{h: pt})

        def proj(db):
            qp = ps([128, R])
            for et in range(8):
                nc.tensor.matmul(qp[:], wq_t[:, et, db * 128:(db + 1) * 128],
                                 xT[:, et, 64:64 + R],
                                 start=(et == 0), stop=(et == 7))
            nc.scalar.add(QT[:, db, :], qp[:], bq_sb[:, db:db + 1])
            kp = ps([128, HALO])
            for et in range(8):
                nc.tensor.matmul(kp[:], wk_t[:, et, db * 128:(db + 1) * 128],
                                 xT[:, et, :], start=(et == 0), stop=(et == 7))
            nc.scalar.add(KT[:, db, :], kp[:], bk_sb[:, db:db + 1])

        def pv_flush(pr):
            db, ptl = pr
            pvs = {}
            for qblk in range(NQB):
                pvs[qblk] = ps([128, 2 * (D + 1)])
            for qblk in range(NQB):
                pv = pvs[qblk]
                for i, h in enumerate((2 * db, 2 * db + 1)):
                    off = i * (D + 1)
                    for cblk in range(2):
                        quad = qblk * 2 + cblk
                        nc.tensor.matmul(pv[:, off:off + D + 1],
                                         ptl[h][:, quad * 128:(quad + 1) * 128],
                                         Vaug[:, qblk + cblk, h, :],
                                         start=(i == 0 and cblk == 0),
                                         stop=False)
            for qblk in range(NQB):
                pv = pvs[qblk]
                nc.tensor.matmul(pv[:, 0:2 * (D + 1)], ones_sb[0:1, :],
                                 biascat[0:1, 2 * db:2 * db + 2, :].rearrange(
                                     "o h d -> o (h d)"),
                                 start=False, stop=True)
            for qblk in range(NQB):
                pv = pvs[qblk]
                zinv = zpool.tile([128, 2], F32, tag="z", name="zinv")
                zsrc = pv[:].rearrange("p (two dd) -> p two dd", dd=D + 1)
                nc.vector.reciprocal(zinv[:], zsrc[:, :, D])
                for i, h in enumerate((2 * db, 2 * db + 1)):
                    off = i * (D + 1)
                    nc.vector.tensor_scalar_mul(
                        Asc[:, qblk, h * D:(h + 1) * D],
                        pv[:, off:off + D], zinv[:, i:i + 1])

        def at_oacc(db):
            # transpose pair db's attention rows and accumulate into O.
            # Runs 2 rounds behind the flush so the PE never waits on the
            # DVE Asc normalization. bv's contribution to O is rank-1 and is
            # folded into the final host-computed bias row (bo + bv@Wo).
            atdb = atpool.tile([128, NQB, 128], BF16, tag="at", name="atdb")
            for qblk in range(NQB):
                tp = ps([128, 128], BF16)
                nc.tensor.transpose(tp[:],
                                    Asc[:, qblk, db * 128:(db + 1) * 128],
                                    identity[:])
                nc.vector.tensor_copy(atdb[:, qblk, :], tp[:])
            for qblk in range(NQB):
                for hf in range(2):
                    nc.tensor.matmul(ops[qblk * 2 + hf][:],
                                     atdb[:, qblk, :],
                                     wo_t[:, db, hf * 512:(hf + 1) * 512],
                                     start=(db == 0), stop=False)

        for r in range(8 + 1):
            if r < 8:
                db = r
                proj(db)
            if r >= 2:
                at_oacc(r - 2)
            if r == 8:
                pv_flush(prev)
            if r < 8:
                if prev is not None:
                    pv_flush(prev)
                ptl = {}
                for i, h in enumerate((2 * db, 2 * db + 1)):
                    rr = i * 64
                    sp = ps([128, 512])
                    for quad in range(4):
                        qblk, cblk = quad // 2, quad % 2
                        nc.tensor.matmul(
                            sp[:, quad * 128:(quad + 1) * 128],
                            KT[rr:rr + 64, db,
                               (qblk + cblk) * 128:(qblk + cblk + 1) * 128],
                            QT[rr:rr + 64, db, qblk * 128:(qblk + 1) * 128],
                            start=(quad == 0), stop=(quad == 3))
                    et_ = epool.tile([128, 512], BF16, tag="e", name="et_")
                    nc.scalar.activation(et_[:], sp[:],
                                         mybir.ActivationFunctionType.Exp)
                    # -1 on the (otherwise idle) gpsimd engine: DVE is the
                    # second-busiest engine during rounds
                    nc.gpsimd.tensor_scalar_add(et_[:], et_[:], -1.0)
                    pt = ppool.tile([128, 512], BF16, tag="p", name="pt")
                    nc.vector.tensor_mul(pt[:], et_[:], m4[:])
                    ptl[h] = pt
                prev = (db, ptl)

        # ---- tail: per-qblk chains of [at_oacc(7) slice, bias row, copy,
        # store] so qblk0's output DMA overlaps qblk1's matmuls. Copies are
        # split across vector and scalar to halve their latency.
        atdb7 = atpool.tile([128, NQB, 128], BF16, tag="at", name="atdb")
        for qblk in range(NQB):
            tp = ps([128, 128], BF16)
            nc.tensor.transpose(tp[:], Asc[:, qblk, 7 * 128:8 * 128],
                                identity[:])
            nc.vector.tensor_copy(atdb7[:, qblk, :], tp[:])
            for hf in range(2):
                nc.tensor.matmul(ops[qblk * 2 + hf][:],
                                 atdb7[:, qblk, :],
                                 wo_t[:, 7, hf * 512:(hf + 1) * 512],
                                 start=False, stop=False)
            for hf in range(2):
                nc.tensor.matmul(ops[qblk * 2 + hf][:], ones_sb[0:1, :],
                                 bo_sb[0:1, hf * 512:(hf + 1) * 512],
                                 start=False, stop=True)
            ob = obpool.tile([128, E], BF16, tag="ob")
            nc.vector.tensor_copy(ob[:, 0:512], ops[qblk * 2][:])
            nc.scalar.copy(ob[:, 512:1024], ops[qblk * 2 + 1][:])
            nc.sync.dma_start(out_d[qblk * 128:(qblk + 1) * 128, :], ob[:])

    nc.compile()
    return nc


_NC = None


def get_nc():
    global _NC
    if _NC is None:
        _NC = build_graph()
    return _NC


def make_in_maps(x, Wq, bq, Wk, bk, Wv, bv, Wo, bo):
    f = lambda a: np.ascontiguousarray(np.asarray(a, dtype=np.float32))
    bf = lambda a: np.ascontiguousarray(
        np.asarray(a, dtype=np.float32).astype(NPBF16))
    x2 = f(x).reshape(N, E)
    Wv32 = f(Wv)
    xsum = x2.sum(0, dtype=np.float32)
    sv = xsum.astype(NPBF16).astype(np.float32) @ Wv32.astype(NPBF16).astype(
        np.float32)  # match on-device bf16 operand rounding
    biascat = np.concatenate(
        [sv.reshape(H, D), np.full((H, 1), float(N), np.float32)],
        axis=1).reshape(1, H * (D + 1))
    ci = np.arange(128, dtype=np.float32)[:, None]  # key index c (partitions)
    qi = np.arange(128, dtype=np.float32)[None, :]  # query index q (free)
    m0 = (ci >= qi).astype(np.float32)
    m1 = (ci <= qi).astype(np.float32)
    mask4 = np.concatenate([m0, m1, m0, m1], axis=1)
    bqk = np.concatenate([f(bq).reshape(8, 128).T,
                          f(bk).reshape(8, 128).T], axis=1)
    # bv's contribution to the output is rank-1: fold bv@Wo into bo
    bo_row = (f(bo) + f(bv) @ f(Wo)).reshape(1, E)
    bcbo = np.concatenate([biascat, bo_row], axis=1)
    common = {
        "Wq": bf(Wq), "Wk": bf(Wk), "Wv": bf(Wv), "Wo": bf(Wo),
        "bqk_r": np.ascontiguousarray(bqk),
        "bcbo": bcbo.astype(NPBF16),
        "ident": np.eye(128, dtype=np.float32).astype(NPBF16),
    }
    in_maps = []
    for c in range(8):
        r0 = c * R
        xh = np.zeros((HALO, E), np.float32)
        lo, hi = r0 - 64, r0 + R + 64
        slo, shi = max(lo, 0), min(hi, N)
        xh[slo - lo: shi - lo] = x2[slo:shi]
        m4c = mask4
        if c == 0:
            m4c = mask4.copy()
            m4c[0:64, 0:128] = 0.0      # quad 0 keys are left padding
        elif c == 7:
            m4c = mask4.copy()
            m4c[64:128, 384:512] = 0.0  # quad 3 keys are right padding
        in_maps.append({**common,
                        "xhT": np.ascontiguousarray(xh.T).astype(NPBF16),
                        "mask4": np.ascontiguousarray(m4c).astype(NPBF16)})
    return in_maps


def kernel(x, Wq, bq, Wk, bk, Wv, bv, Wo, bo, _trace=False, _trace_kwargs=None):
    nc = get_nc()
    in_maps = make_in_maps(x, Wq, bq, Wk, bk, Wv, bv, Wo, bo)
    res = run_bass_kernel_spmd(nc, in_maps, list(range(8)), trace=_trace,
                               **(_trace_kwargs or {}))
    out = np.concatenate([np.asarray(res.results[c]["out"]) for c in range(8)],
                         axis=0)
    kernel.last_result = res
    return out[None].astype(np.float32)


# revision 37
# speedup vs baseline: 2.6097x; 2.6097x over previous
"""Multi-head dilated sliding-window attention (window=129, dil=1) on 8 TRN2 cores.

Sharding: sequence-parallel. Each core computes 256 query rows (N=2048 / 8),
with a 64-row K/V halo on each side (zero-padded at the sequence edges).
Weights are replicated (resident in SBUF, bf16).

Band-softmax identity used (reference softmaxes the FULL row with zeros
outside the band):
    out_i = (sum_band (e^{s_ij} - 1) V_j + sum_all V_j) / (sum_band (e^{s_ij} - 1) + N)
computed per head with V_raw = x@Wv (no bias; bv is folded in after the
attention average). bk is applied unconditionally as a per-partition scalar
bias; contributions from zero-padded halo keys (which would wrongly score
e^{q.bk}-1 != 0) are killed by per-core masks that zero those key rows.
The global row sum_all V_j = (sum_n x_n) @ Wv and its +N denominator count
are precomputed on the host into `biascat`.

Compute dtype: bf16 operands into the PE (fp32 runs at quarter rate on TRN2),
fp32 PSUM accumulation; the exp/-1/mask chain runs in bf16 on ACT/DVE (2x DVE
rate; P is stored bf16 anyway so this costs no extra error).

Structure: Q^T/K^T projections are computed per head-pair (db) and attention
for that pair runs immediately, pipelined one round behind the scores so the
PE never stalls on the ACT/DVE softmax chain. The output projection
O = (A + bv) @ Wo is folded into the same pipeline: each round transposes the
previous pair's attention rows and accumulates A_db @ Wo[db-slice] into 4
persistent PSUM banks, so only the bias row + output DMA remain at the end.
"""

import numpy as np
import ml_dtypes
from contextlib import ExitStack

import concourse.bass as bass
import concourse.tile as tile
from concourse import bacc, mybir
from concourse.bass_utils import run_bass_kernel_spmd

F32 = mybir.dt.float32
BF16 = mybir.dt.bfloat16
NPBF16 = ml_dtypes.bfloat16
N, E, H, D = 2048, 1024, 16, 64
R = N // 8          # 256 query rows per core
HALO = R + 128      # 384 K/V rows per core
NQB = R // 128      # query blocks per core


def build_graph():
    nc = bacc.Bacc("TRN2", target_bir_lowering=False, debug=False, num_devices=8)

    xh_d = nc.declare_dram_parameter("xhT", [E, HALO], BF16, isOutput=False)
    wq_d = nc.declare_dram_parameter("Wq", [E, H * D], BF16, isOutput=False)
    wk_d = nc.declare_dram_parameter("Wk", [E, H * D], BF16, isOutput=False)
    wv_d = nc.declare_dram_parameter("Wv", [E, H * D], BF16, isOutput=False)
    wo_d = nc.declare_dram_parameter("Wo", [H * D, E], BF16, isOutput=False)
    bqk_d = nc.declare_dram_parameter("bqk_r", [128, 16], F32, isOutput=False)
    bcbo_d = nc.declare_dram_parameter("bcbo", [1, H * (D + 1) + E], BF16,
                                       isOutput=False)
    m4_d = nc.declare_dram_parameter("mask4", [128, 512], BF16, isOutput=False)
    id_d = nc.declare_dram_parameter("ident", [128, 128], BF16, isOutput=False)
    out_d = nc.declare_dram_parameter("out", [R, E], BF16, isOutput=True)

    with tile.TileContext(nc) as tc, ExitStack() as ctx:
        const = ctx.enter_context(tc.tile_pool(name="const", bufs=1))
        pers = ctx.enter_context(tc.tile_pool(name="pers", bufs=1))
        epool = ctx.enter_context(tc.tile_pool(name="epool", bufs=3))
        ppool = ctx.enter_context(tc.tile_pool(name="ppool", bufs=5))
        zpool = ctx.enter_context(tc.tile_pool(name="zpool", bufs=4))
        atpool = ctx.enter_context(tc.tile_pool(name="atpool", bufs=3))
        obpool = ctx.enter_context(tc.tile_pool(name="obpool", bufs=2))
        psum = ctx.enter_context(tc.tile_pool(name="psum", bufs=4, space="PSUM"))
        opsum = ctx.enter_context(tc.tile_pool(name="opsum", bufs=1, space="PSUM"))

        def ps(shape, dt=F32):
            return psum.tile(shape, dt, tag="ps", name="pst")

        # ---- loads. A dma_start costs ~607ns of issue time on the Sync
        # engine REGARDLESS of size, and each transfer's packets spray
        # round-robin over all 16 DMA engines (~432 GB/s aggregate). So:
        # fewest possible dma_starts, ordered by first use; weights are
        # split in two chunks only so consumers can start on the first half.
        # x arrives pre-transposed from the host: [e_p, e_t, seq]
        xT = pers.tile([128, 8, HALO], BF16, tag="xT")
        identity = const.tile([128, 128], BF16, tag="identity")

        def wtile(nm):
            return const.tile([128, 8, E], BF16, tag=nm, name="wt")

        wv_t = wtile("wv")
        wq_t = wtile("wq")
        wk_t = wtile("wk")
        wo_t = wtile("wo")
        m4 = const.tile([128, 512], BF16, tag="m4")
        bqk_sb = const.tile([128, 16], F32, tag="bqk")
        bq_sb = bqk_sb[:, 0:8]
        bk_sb = bqk_sb[:, 8:16]
        bcbo_sb = const.tile([1, H * (D + 1) + E], BF16, tag="bcbo")
        biascat = bcbo_sb[:, 0:H * (D + 1)].rearrange(
            "o (h d) -> o h d", d=D + 1)
        bo_sb = bcbo_sb[:, H * (D + 1):]

        # DMA order = v9 tuning: xhT whole, wv in 4 chunks (V ramps the PE
        # clock on its DMA-paced prefix, which also keeps early activity
        # density below the HAM duty-throttle trip point), then wq/wk/wo.
        # Starting V earlier (interleaved xhT/wv chunks) measured WORSE:
        # the denser early activity trips a 4/8 duty clamp onto rounds 0-1.
        nc.sync.dma_start(
            xT[:], xh_d[:, :].rearrange("(et p) s -> p et s", p=128))
        nc.sync.dma_start(identity[:], id_d[:, :])
        wvsrc = wv_d[:, :].rearrange("(et p) c -> p et c", p=128)
        for ch in range(4):
            nc.sync.dma_start(wv_t[:, 2 * ch:2 * ch + 2, :],
                              wvsrc[:, 2 * ch:2 * ch + 2, :])
        nc.sync.dma_start(m4[:], m4_d[:, :])
        nc.sync.dma_start(bqk_sb[:], bqk_d[:, :])
        nc.sync.dma_start(bcbo_sb[:], bcbo_d[:, :])

        def wdma(wt, src_d):
            src = src_d[:, :].rearrange("(et p) c -> p et c", p=128)
            nc.sync.dma_start(wt[:, 0:4, :], src[:, 0:4, :])
            nc.sync.dma_start(wt[:, 4:8, :], src[:, 4:8, :])

        wdma(wq_t, wq_d)
        wdma(wk_t, wk_d)
        wdma(wo_t, wo_d)
        ones_sb = const.tile([1, 128], BF16, tag="ones")
        nc.vector.memset(ones_sb[:], 1.0)

        # ---- persistent activations ---------------------------------------
        QT = pers.tile([128, 8, R], BF16, tag="QT")          # [d_p, d_t, q]
        KT = pers.tile([128, 8, HALO], BF16, tag="KT")       # [d_p, d_t, seq]
        Vaug = pers.tile([128, 3, H, D + 1], BF16, tag="Vaug")
        Asc = pers.tile([128, NQB, H * D], BF16, tag="Asc")  # [q_p, qblk, dims]

        # ---- persistent O-projection accumulators (4 PSUM banks) ----------
        ops = [opsum.tile([128, 512], F32, tag=f"ops{i}", name=f"ops{i}")
               for i in range(2 * NQB)]

        # ---- V (natural layout, raw): st-serial so only 2 PSUM banks are
        # held, letting the first attention rounds overlap later V blocks.
        for st in range(3):
            vp = [ps([128, 512]) for _ in range(2)]
            for et in range(8):
                for hf in range(2):
                    nc.tensor.matmul(vp[hf][:],
                                     xT[:, et, st * 128:(st + 1) * 128],
                                     wv_t[:, et, hf * 512:(hf + 1) * 512],
                                     start=(et == 0), stop=(et == 7))
            for hf in range(2):
                src = vp[hf][:].rearrange("p (h d) -> p h d", d=D)
                nc.scalar.copy(Vaug[:, st, hf * 8:(hf + 1) * 8, 0:D], src)
        nc.vector.memset(Vaug[:, :, :, D:D + 1], 1.0)

        # ---- fused projections + banded attention + O accumulation, one
        # head-pair at a time. Round r = db. Emission order:
        #   1. Q^T/K^T projection matmuls for db
        #   2. PV + bias matmuls, epilogue, A-transpose and O-accumulation
        #      of round r-1 (p tiles ready)
        #   3. S matmuls (one [128, 512] psum per head = both qblk/cblk
        #      quadrants) + exp/-1/mask chain for round r
        # Per-head p layout: [q0c0 | q0c1 | q1c0 | q1c1], quadrant j uses
        # keys halo block (qblk+cblk) and mask m0/m1 alternating.
        prev = None  # (db, ptiles{h: pt})

        def proj(db):
            qp = ps([128, R])
            for et in range(8):
                nc.tensor.matmul(qp[:], wq_t[:, et, db * 128:(db + 1) * 128],
                                 xT[:, et, 64:64 + R],
                                 start=(et == 0), stop=(et == 7))
            nc.scalar.add(QT[:, db, :], qp[:], bq_sb[:, db:db + 1])
            kp = ps([128, HALO])
            for et in range(8):
                nc.tensor.matmul(kp[:], wk_t[:, et, db * 128:(db + 1) * 128],
                                 xT[:, et, :], start=(et == 0), stop=(et == 7))
            nc.scalar.add(KT[:, db, :], kp[:], bk_sb[:, db:db + 1])

        def pv_flush(pr):
            db, ptl = pr
            pvs = {}
            for qblk in range(NQB):
                pvs[qblk] = ps([128, 2 * (D + 1)])
            for qblk in range(NQB):
                pv = pvs[qblk]
                for i, h in enumerate((2 * db, 2 * db + 1)):
                    off = i * (D + 1)
                    for cblk in range(2):
                        quad = qblk * 2 + cblk
                        nc.tensor.matmul(pv[:, off:off + D + 1],
                                         ptl[h][:, quad * 128:(quad + 1) * 128],
                                         Vaug[:, qblk + cblk, h, :],
                                         start=(i == 0 and cblk == 0),
                                         stop=False)
            for qblk in range(NQB):
                pv = pvs[qblk]
                nc.tensor.matmul(pv[:, 0:2 * (D + 1)], ones_sb[0:1, :],
                                 biascat[0:1, 2 * db:2 * db + 2, :].rearrange(
                                     "o h d -> o (h d)"),
                                 start=False, stop=True)
            for qblk in range(NQB):
                pv = pvs[qblk]
                zinv = zpool.tile([128, 2], F32, tag="z", name="zinv")
                zsrc = pv[:].rearrange("p (two dd) -> p two dd", dd=D + 1)
                nc.vector.reciprocal(zinv[:], zsrc[:, :, D])
                for i, h in enumerate((2 * db, 2 * db + 1)):
                    off = i * (D + 1)
                    nc.vector.tensor_scalar_mul(
                        Asc[:, qblk, h * D:(h + 1) * D],
                        pv[:, off:off + D], zinv[:, i:i + 1])

        def at_oacc(db):
            # transpose pair db's attention rows and accumulate into O.
            # Runs 2 rounds behind the flush so the PE never waits on the
            # DVE Asc normalization. bv's contribution to O is rank-1 and is
            # folded into the final host-computed bias row (bo + bv@Wo).
            atdb = atpool.tile([128, NQB, 128], BF16, tag="at", name="atdb")
            for qblk in range(NQB):
                tp = ps([128, 128], BF16)
                nc.tensor.transpose(tp[:],
                                    Asc[:, qblk, db * 128:(db + 1) * 128],
                                    identity[:])
                nc.vector.tensor_copy(atdb[:, qblk, :], tp[:])
            for qblk in range(NQB):
                for hf in range(2):
                    nc.tensor.matmul(ops[qblk * 2 + hf][:],
                                     atdb[:, qblk, :],
                                     wo_t[:, db, hf * 512:(hf + 1) * 512],
                                     start=(db == 0), stop=False)

        for r in range(8 + 1):
            if r < 8:
                db = r
                proj(db)
            if r >= 2:
                at_oacc(r - 2)
            if r == 8:
                pv_flush(prev)
            if r < 8:
                if prev is not None:
                    pv_flush(prev)
                ptl = {}
                for i, h in enumerate((2 * db, 2 * db + 1)):
                    rr = i * 64
                    sp = ps([128, 512])
                    for quad in range(4):
                        qblk, cblk = quad // 2, quad % 2
                        nc.tensor.matmul(
                            sp[:, quad * 128:(quad + 1) * 128],
                            KT[rr:rr + 64, db,
                               (qblk + cblk) * 128:(qblk + cblk + 1) * 128],
                            QT[rr:rr + 64, db, qblk * 128:(qblk + 1) * 128],
                            start=(quad == 0), stop=(quad == 3))
                    et_ = epool.tile([128, 512], BF16, tag="e", name="et_")
                    nc.scalar.activation(et_[:], sp[:],
                                         mybir.ActivationFunctionType.Exp)
                    # NOTE: tried gpsimd here to unload DVE — catastrophic
                    # (~2.6x whole-kernel slowdown); Pool tensor ops are slow
                    nc.vector.tensor_scalar_add(et_[:], et_[:], -1.0)
                    pt = ppool.tile([128, 512], BF16, tag="p", name="pt")
                    nc.vector.tensor_mul(pt[:], et_[:], m4[:])
                    ptl[h] = pt
                prev = (db, ptl)

        # ---- tail: per-qblk chains of [at_oacc(7) slice, bias row, copy,
        # store] so qblk0's output DMA overlaps qblk1's matmuls. Copies are
        # split across vector and scalar to halve their latency.
        atdb7 = atpool.tile([128, NQB, 128], BF16, tag="at", name="atdb")
        for qblk in range(NQB):
            tp = ps([128, 128], BF16)
            nc.tensor.transpose(tp[:], Asc[:, qblk, 7 * 128:8 * 128],
                                identity[:])
            nc.vector.tensor_copy(atdb7[:, qblk, :], tp[:])
            for hf in range(2):
                nc.tensor.matmul(ops[qblk * 2 + hf][:],
                                 atdb7[:, qblk, :],
                                 wo_t[:, 7, hf * 512:(hf + 1) * 512],
                                 start=False, stop=False)
            for hf in range(2):
                nc.tensor.matmul(ops[qblk * 2 + hf][:], ones_sb[0:1, :],
                                 bo_sb[0:1, hf * 512:(hf + 1) * 512],
                                 start=False, stop=True)
            ob = obpool.tile([128, E], BF16, tag="ob")
            nc.vector.tensor_copy(ob[:, 0:512], ops[qblk * 2][:])
            nc.scalar.copy(ob[:, 512:1024], ops[qblk * 2 + 1][:])
            nc.sync.dma_start(out_d[qblk * 128:(qblk + 1) * 128, :], ob[:])

    nc.compile()
    return nc


_NC = None


def get_nc():
    global _NC
    if _NC is None:
        _NC = build_graph()
    return _NC


def make_in_maps(x, Wq, bq, Wk, bk, Wv, bv, Wo, bo):
    f = lambda a: np.ascontiguousarray(np.asarray(a, dtype=np.float32))
    bf = lambda a: np.ascontiguousarray(
        np.asarray(a, dtype=np.float32).astype(NPBF16))
    x2 = f(x).reshape(N, E)
    Wv32 = f(Wv)
    xsum = x2.sum(0, dtype=np.float32)
    sv = xsum.astype(NPBF16).astype(np.float32) @ Wv32.astype(NPBF16).astype(
        np.float32)  # match on-device bf16 operand rounding
    biascat = np.concatenate(
        [sv.reshape(H, D), np.full((H, 1), float(N), np.float32)],
        axis=1).reshape(1, H * (D + 1))
    ci = np.arange(128, dtype=np.float32)[:, None]  # key index c (partitions)
    qi = np.arange(128, dtype=np.float32)[None, :]  # query index q (free)
    m0 = (ci >= qi).astype(np.float32)
    m1 = (ci <= qi).astype(np.float32)
    mask4 = np.concatenate([m0, m1, m0, m1], axis=1)
    bqk = np.concatenate([f(bq).reshape(8, 128).T,
                          f(bk).reshape(8, 128).T], axis=1)
    # bv's contribution to the output is rank-1: fold bv@Wo into bo
    bo_row = (f(bo) + f(bv) @ f(Wo)).reshape(1, E)
    bcbo = np.concatenate([biascat, bo_row], axis=1)
    common = {
        "Wq": bf(Wq), "Wk": bf(Wk), "Wv": bf(Wv), "Wo": bf(Wo),
        "bqk_r": np.ascontiguousarray(bqk),
        "bcbo": bcbo.astype(NPBF16),
        "ident": np.eye(128, dtype=np.float32).astype(NPBF16),
    }
    in_maps = []
    for c in range(8):
        r0 = c * R
        xh = np.zeros((HALO, E), np.float32)
        lo, hi = r0 - 64, r0 + R + 64
        slo, shi = max(lo, 0), min(hi, N)
        xh[slo - lo: shi - lo] = x2[slo:shi]
        m4c = mask4
        if c == 0:
            m4c = mask4.copy()
            m4c[0:64, 0:128] = 0.0      # quad 0 keys are left padding
        elif c == 7:
            m4c = mask4.copy()
            m4c[64:128, 384:512] = 0.0  # quad 3 keys are right padding
        in_maps.append({**common,
                        "xhT": np.ascontiguousarray(xh.T).astype(NPBF16),
                        "mask4": np.ascontiguousarray(m4c).astype(NPBF16)})
    return in_maps


def kernel(x, Wq, bq, Wk, bk, Wv, bv, Wo, bo, _trace=False, _trace_kwargs=None):
    nc = get_nc()
    in_maps = make_in_maps(x, Wq, bq, Wk, bk, Wv, bv, Wo, bo)
    res = run_bass_kernel_spmd(nc, in_maps, list(range(8)), trace=_trace,
                               **(_trace_kwargs or {}))
    out = np.concatenate([np.asarray(res.results[c]["out"]) for c in range(8)],
                         axis=0)
    kernel.last_result = res
    return out[None].astype(np.float32)


# revision 40
# speedup vs baseline: 2.6117x; 1.0008x over previous
"""Multi-head dilated sliding-window attention (window=129, dil=1) on 8 TRN2 cores.

Sharding: sequence-parallel. Each core computes 256 query rows (N=2048 / 8),
with a 64-row K/V halo on each side (zero-padded at the sequence edges).
Weights are replicated (resident in SBUF, bf16).

Band-softmax identity used (reference softmaxes the FULL row with zeros
outside the band):
    out_i = (sum_band (e^{s_ij} - 1) V_j + sum_all V_j) / (sum_band (e^{s_ij} - 1) + N)
computed per head with V_raw = x@Wv (no bias; bv is folded in after the
attention average). bk is applied unconditionally as a per-partition scalar
bias; contributions from zero-padded halo keys (which would wrongly score
e^{q.bk}-1 != 0) are killed by per-core masks that zero those key rows.
The global row sum_all V_j = (sum_n x_n) @ Wv and its +N denominator count
are precomputed on the host into `biascat`.

Compute dtype: bf16 operands into the PE (fp32 runs at quarter rate on TRN2),
fp32 PSUM accumulation; the exp/-1/mask chain runs in bf16 on ACT/DVE (2x DVE
rate; P is stored bf16 anyway so this costs no extra error).

Structure: Q^T/K^T projections are computed per head-pair (db) and attention
for that pair runs immediately, pipelined one round behind the scores so the
PE never stalls on the ACT/DVE softmax chain. The output projection
O = (A + bv) @ Wo is folded into the same pipeline: each round transposes the
previous pair's attention rows and accumulates A_db @ Wo[db-slice] into 4
persistent PSUM banks, so only the bias row + output DMA remain at the end.
"""

import numpy as np
import ml_dtypes
from contextlib import ExitStack

import concourse.bass as bass
import concourse.tile as tile
from concourse import bacc, mybir
from concourse.bass_utils import run_bass_kernel_spmd

F32 = mybir.dt.float32
BF16 = mybir.dt.bfloat16
NPBF16 = ml_dtypes.bfloat16
N, E, H, D = 2048, 1024, 16, 64
R = N // 8          # 256 query rows per core
HALO = R + 128      # 384 K/V rows per core
NQB = R // 128      # query blocks per core


def build_graph():
    nc = bacc.Bacc("TRN2", target_bir_lowering=False, debug=False, num_devices=8)

    xh_d = nc.declare_dram_parameter("xhT", [E, HALO], BF16, isOutput=False)
    wq_d = nc.declare_dram_parameter("Wq", [E, H * D], BF16, isOutput=False)
    wk_d = nc.declare_dram_parameter("Wk", [E, H * D], BF16, isOutput=False)
    wv_d = nc.declare_dram_parameter("Wv", [E, H * D], BF16, isOutput=False)
    wo_d = nc.declare_dram_parameter("Wo", [H * D, E], BF16, isOutput=False)
    bqk_d = nc.declare_dram_parameter("bqk_r", [128, 16], F32, isOutput=False)
    bcbo_d = nc.declare_dram_parameter("bcbo", [1, H * (D + 1) + E], BF16,
                                       isOutput=False)
    m4_d = nc.declare_dram_parameter("mask4", [128, 512], BF16, isOutput=False)
    id_d = nc.declare_dram_parameter("ident", [128, 128], BF16, isOutput=False)
    out_d = nc.declare_dram_parameter("out", [R, E], BF16, isOutput=True)

    with tile.TileContext(nc) as tc, ExitStack() as ctx:
        const = ctx.enter_context(tc.tile_pool(name="const", bufs=1))
        pers = ctx.enter_context(tc.tile_pool(name="pers", bufs=1))
        epool = ctx.enter_context(tc.tile_pool(name="epool", bufs=3))
        ppool = ctx.enter_context(tc.tile_pool(name="ppool", bufs=5))
        zpool = ctx.enter_context(tc.tile_pool(name="zpool", bufs=4))
        atpool = ctx.enter_context(tc.tile_pool(name="atpool", bufs=3))
        obpool = ctx.enter_context(tc.tile_pool(name="obpool", bufs=2))
        psum = ctx.enter_context(tc.tile_pool(name="psum", bufs=4, space="PSUM"))
        opsum = ctx.enter_context(tc.tile_pool(name="opsum", bufs=1, space="PSUM"))

        def ps(shape, dt=F32):
            return psum.tile(shape, dt, tag="ps", name="pst")

        # ---- loads. A dma_start costs ~607ns of issue time on the Sync
        # engine REGARDLESS of size, and each transfer's packets spray
        # round-robin over all 16 DMA engines (~432 GB/s aggregate). So:
        # fewest possible dma_starts, ordered by first use; weights are
        # split in two chunks only so consumers can start on the first half.
        # x arrives pre-transposed from the host: [e_p, e_t, seq]
        xT = pers.tile([128, 8, HALO], BF16, tag="xT")
        identity = const.tile([128, 128], BF16, tag="identity")

        def wtile(nm):
            return const.tile([128, 8, E], BF16, tag=nm, name="wt")

        wv_t = wtile("wv")
        wq_t = wtile("wq")
        wk_t = wtile("wk")
        wo_t = wtile("wo")
        m4 = const.tile([128, 512], BF16, tag="m4")
        bqk_sb = const.tile([128, 16], F32, tag="bqk")
        bq_sb = bqk_sb[:, 0:8]
        bk_sb = bqk_sb[:, 8:16]
        bcbo_sb = const.tile([1, H * (D + 1) + E], BF16, tag="bcbo")
        biascat = bcbo_sb[:, 0:H * (D + 1)].rearrange(
            "o (h d) -> o h d", d=D + 1)
        bo_sb = bcbo_sb[:, H * (D + 1):]

        # DMA order = v9 tuning: xhT whole, wv in 4 chunks (V ramps the PE
        # clock on its DMA-paced prefix, which also keeps early activity
        # density below the HAM duty-throttle trip point), then wq/wk/wo.
        # Starting V earlier (interleaved xhT/wv chunks) measured WORSE:
        # the denser early activity trips a 4/8 duty clamp onto rounds 0-1.
        nc.sync.dma_start(
            xT[:], xh_d[:, :].rearrange("(et p) s -> p et s", p=128))
        nc.sync.dma_start(identity[:], id_d[:, :])
        wvsrc = wv_d[:, :].rearrange("(et p) c -> p et c", p=128)
        for ch in range(4):
            nc.sync.dma_start(wv_t[:, 2 * ch:2 * ch + 2, :],
                              wvsrc[:, 2 * ch:2 * ch + 2, :])
        nc.sync.dma_start(m4[:], m4_d[:, :])
        nc.sync.dma_start(bqk_sb[:], bqk_d[:, :])
        nc.sync.dma_start(bcbo_sb[:], bcbo_d[:, :])

        def wdma(wt, src_d):
            src = src_d[:, :].rearrange("(et p) c -> p et c", p=128)
            nc.sync.dma_start(wt[:, 0:4, :], src[:, 0:4, :])
            nc.sync.dma_start(wt[:, 4:8, :], src[:, 4:8, :])

        wdma(wq_t, wq_d)
        wdma(wk_t, wk_d)
        wdma(wo_t, wo_d)
        ones_sb = const.tile([1, 128], BF16, tag="ones")
        nc.vector.memset(ones_sb[:], 1.0)

        # ---- persistent activations ---------------------------------------
        QT = pers.tile([128, 8, R], BF16, tag="QT")          # [d_p, d_t, q]
        KT = pers.tile([128, 8, HALO], BF16, tag="KT")       # [d_p, d_t, seq]
        Vaug = pers.tile([128, 3, H, D + 1], BF16, tag="Vaug")
        Asc = pers.tile([128, NQB, H * D], BF16, tag="Asc")  # [q_p, qblk, dims]

        # ---- persistent O-projection accumulators (4 PSUM banks; a
        # matmul dst cannot cross a 2KB psum bank, so 4x [128,512]) --------
        ops = [opsum.tile([128, 512], F32, tag=f"ops{i}", name=f"ops{i}")
               for i in range(2 * NQB)]

        # ---- V (natural layout, raw): st-serial so only 2 PSUM banks are
        # held, letting the first attention rounds overlap later V blocks.
        for st in range(3):
            vp = [ps([128, 512]) for _ in range(2)]
            for et in range(8):
                for hf in range(2):
                    nc.tensor.matmul(vp[hf][:],
                                     xT[:, et, st * 128:(st + 1) * 128],
                                     wv_t[:, et, hf * 512:(hf + 1) * 512],
                                     start=(et == 0), stop=(et == 7))
            for hf in range(2):
                src = vp[hf][:].rearrange("p (h d) -> p h d", d=D)
                nc.scalar.copy(Vaug[:, st, hf * 8:(hf + 1) * 8, 0:D], src)
        nc.vector.memset(Vaug[:, :, :, D:D + 1], 1.0)

        # ---- fused projections + banded attention + O accumulation, one
        # head-pair at a time. Round r = db. Emission order:
        #   1. Q^T/K^T projection matmuls for db
        #   2. PV + bias matmuls, epilogue, A-transpose and O-accumulation
        #      of round r-1 (p tiles ready)
        #   3. S matmuls (one [128, 512] psum per head = both qblk/cblk
        #      quadrants) + exp/-1/mask chain for round r
        # Per-head p layout: [q0c0 | q0c1 | q1c0 | q1c1], quadrant j uses
        # keys halo block (qblk+cblk) and mask m0/m1 alternating.
        prev = None  # (db, ptiles{h: pt})

        def proj(db):
            qp = ps([128, R])
            for et in range(8):
                nc.tensor.matmul(qp[:], wq_t[:, et, db * 128:(db + 1) * 128],
                                 xT[:, et, 64:64 + R],
                                 start=(et == 0), stop=(et == 7))
            nc.scalar.add(QT[:, db, :], qp[:], bq_sb[:, db:db + 1])
            kp = ps([128, HALO])
            for et in range(8):
                nc.tensor.matmul(kp[:], wk_t[:, et, db * 128:(db + 1) * 128],
                                 xT[:, et, :], start=(et == 0), stop=(et == 7))
            nc.scalar.add(KT[:, db, :], kp[:], bk_sb[:, db:db + 1])

        def pv_flush(pr):
            db, ptl = pr
            pvs = {}
            for qblk in range(NQB):
                pvs[qblk] = ps([128, 2 * (D + 1)])
            for qblk in range(NQB):
                pv = pvs[qblk]
                for i, h in enumerate((2 * db, 2 * db + 1)):
                    off = i * (D + 1)
                    for cblk in range(2):
                        quad = qblk * 2 + cblk
                        nc.tensor.matmul(pv[:, off:off + D + 1],
                                         ptl[h][:, quad * 128:(quad + 1) * 128],
                                         Vaug[:, qblk + cblk, h, :],
                                         start=(i == 0 and cblk == 0),
                                         stop=False)
            for qblk in range(NQB):
                pv = pvs[qblk]
                nc.tensor.matmul(pv[:, 0:2 * (D + 1)], ones_sb[0:1, :],
                                 biascat[0:1, 2 * db:2 * db + 2, :].rearrange(
                                     "o h d -> o (h d)"),
                                 start=False, stop=True)
            for qblk in range(NQB):
                pv = pvs[qblk]
                zinv = zpool.tile([128, 2], F32, tag="z", name="zinv")
                zsrc = pv[:].rearrange("p (two dd) -> p two dd", dd=D + 1)
                nc.vector.reciprocal(zinv[:], zsrc[:, :, D])
                for i, h in enumerate((2 * db, 2 * db + 1)):
                    off = i * (D + 1)
                    nc.vector.tensor_scalar_mul(
                        Asc[:, qblk, h * D:(h + 1) * D],
                        pv[:, off:off + D], zinv[:, i:i + 1])

        def at_oacc(db):
            # transpose pair db's attention rows and accumulate into O.
            # Runs 2 rounds behind the flush so the PE never waits on the
            # DVE Asc normalization. bv's contribution to O is rank-1 and is
            # folded into the final host-computed bias row (bo + bv@Wo).
            atdb = atpool.tile([128, NQB, 128], BF16, tag="at", name="atdb")
            for qblk in range(NQB):
                tp = ps([128, 128], BF16)
                nc.tensor.transpose(tp[:],
                                    Asc[:, qblk, db * 128:(db + 1) * 128],
                                    identity[:])
                nc.vector.tensor_copy(atdb[:, qblk, :], tp[:])
            for qblk in range(NQB):
                for hf in range(2):
                    nc.tensor.matmul(ops[qblk * 2 + hf][:],
                                     atdb[:, qblk, :],
                                     wo_t[:, db, hf * 512:(hf + 1) * 512],
                                     start=(db == 0), stop=False)

        for r in range(8 + 1):
            if r < 8:
                db = r
                proj(db)
            if r >= 2:
                at_oacc(r - 2)
            if r == 8:
                pv_flush(prev)
            if r < 8:
                if prev is not None:
                    pv_flush(prev)
                ptl = {}
                for i, h in enumerate((2 * db, 2 * db + 1)):
                    rr = i * 64
                    sp = ps([128, 512])
                    # quads 1 and 2 share the key block (1) and their query
                    # ranges and dst are contiguous -> one 256-free matmul
                    nc.tensor.matmul(
                        sp[:, 0:128], KT[rr:rr + 64, db, 0:128],
                        QT[rr:rr + 64, db, 0:128], start=True, stop=False)
                    nc.tensor.matmul(
                        sp[:, 128:384], KT[rr:rr + 64, db, 128:256],
                        QT[rr:rr + 64, db, 0:256], start=False, stop=False)
                    nc.tensor.matmul(
                        sp[:, 384:512], KT[rr:rr + 64, db, 256:384],
                        QT[rr:rr + 64, db, 128:256], start=False, stop=True)
                    et_ = epool.tile([128, 512], BF16, tag="e", name="et_")
                    nc.scalar.activation(et_[:], sp[:],
                                         mybir.ActivationFunctionType.Exp)
                    # NOTE: tried gpsimd here to unload DVE — catastrophic
                    # (~2.6x whole-kernel slowdown); Pool tensor ops are slow
                    nc.vector.tensor_scalar_add(et_[:], et_[:], -1.0)
                    pt = ppool.tile([128, 512], BF16, tag="p", name="pt")
                    nc.vector.tensor_mul(pt[:], et_[:], m4[:])
                    ptl[h] = pt
                prev = (db, ptl)

        # ---- tail: per-qblk chains of [at_oacc(7) slice, bias row, copy,
        # store] so qblk0's output DMA overlaps qblk1's matmuls. Copies are
        # split across vector and scalar to halve their latency.
        atdb7 = atpool.tile([128, NQB, 128], BF16, tag="at", name="atdb")
        for qblk in range(NQB):
            tp = ps([128, 128], BF16)
            nc.tensor.transpose(tp[:], Asc[:, qblk, 7 * 128:8 * 128],
                                identity[:])
            nc.vector.tensor_copy(atdb7[:, qblk, :], tp[:])
            for hf in range(2):
                nc.tensor.matmul(ops[qblk * 2 + hf][:], atdb7[:, qblk, :],
                                 wo_t[:, 7, hf * 512:(hf + 1) * 512],
                                 start=False, stop=False)
            for hf in range(2):
                nc.tensor.matmul(ops[qblk * 2 + hf][:], ones_sb[0:1, :],
                                 bo_sb[0:1, hf * 512:(hf + 1) * 512],
                                 start=False, stop=True)
            ob = obpool.tile([128, E], BF16, tag="ob")
            nc.vector.tensor_copy(ob[:, 0:512], ops[qblk * 2][:])
            nc.scalar.copy(ob[:, 512:1024], ops[qblk * 2 + 1][:])
            nc.sync.dma_start(out_d[qblk * 128:(qblk + 1) * 128, :], ob[:])

    nc.compile()
    return nc


_NC = None


def get_nc():
    global _NC
    if _NC is None:
        _NC = build_graph()
    return _NC


def make_in_maps(x, Wq, bq, Wk, bk, Wv, bv, Wo, bo):
    f = lambda a: np.ascontiguousarray(np.asarray(a, dtype=np.float32))
    bf = lambda a: np.ascontiguousarray(
        np.asarray(a, dtype=np.float32).astype(NPBF16))
    x2 = f(x).reshape(N, E)
    Wv32 = f(Wv)
    xsum = x2.sum(0, dtype=np.float32)
    sv = xsum.astype(NPBF16).astype(np.float32) @ Wv32.astype(NPBF16).astype(
        np.float32)  # match on-device bf16 operand rounding
    biascat = np.concatenate(
        [sv.reshape(H, D), np.full((H, 1), float(N), np.float32)],
        axis=1).reshape(1, H * (D + 1))
    ci = np.arange(128, dtype=np.float32)[:, None]  # key index c (partitions)
    qi = np.arange(128, dtype=np.float32)[None, :]  # query index q (free)
    m0 = (ci >= qi).astype(np.float32)
    m1 = (ci <= qi).astype(np.float32)
    mask4 = np.concatenate([m0, m1, m0, m1], axis=1)
    bqk = np.concatenate([f(bq).reshape(8, 128).T,
                          f(bk).reshape(8, 128).T], axis=1)
    # bv's contribution to the output is rank-1: fold bv@Wo into bo
    bo_row = (f(bo) + f(bv) @ f(Wo)).reshape(1, E)
    bcbo = np.concatenate([biascat, bo_row], axis=1)
    common = {
        "Wq": bf(Wq), "Wk": bf(Wk), "Wv": bf(Wv), "Wo": bf(Wo),
        "bqk_r": np.ascontiguousarray(bqk),
        "bcbo": bcbo.astype(NPBF16),
        "ident": np.eye(128, dtype=np.float32).astype(NPBF16),
    }
    in_maps = []
    for c in range(8):
        r0 = c * R
        xh = np.zeros((HALO, E), np.float32)
        lo, hi = r0 - 64, r0 + R + 64
        slo, shi = max(lo, 0), min(hi, N)
        xh[slo - lo: shi - lo] = x2[slo:shi]
        m4c = mask4
        if c == 0:
            m4c = mask4.copy()
            m4c[0:64, 0:128] = 0.0      # quad 0 keys are left padding
        elif c == 7:
            m4c = mask4.copy()
            m4c[64:128, 384:512] = 0.0  # quad 3 keys are right padding
        in_maps.append({**common,
                        "xhT": np.ascontiguousarray(xh.T).astype(NPBF16),
                        "mask4": np.ascontiguousarray(m4c).astype(NPBF16)})
    return in_maps


def kernel(x, Wq, bq, Wk, bk, Wv, bv, Wo, bo, _trace=False, _trace_kwargs=None):
    nc = get_nc()
    in_maps = make_in_maps(x, Wq, bq, Wk, bk, Wv, bv, Wo, bo)
    res = run_bass_kernel_spmd(nc, in_maps, list(range(8)), trace=_trace,
                               **(_trace_kwargs or {}))
    out = np.concatenate([np.asarray(res.results[c]["out"]) for c in range(8)],
                         axis=0)
    kernel.last_result = res
    return out[None].astype(np.float32)
